# revision 15
# baseline (speedup 1.0000x reference)
"""2-layer GCN + edge-logit decoder on 8 Trainium2 NeuronCores.

v2: scatter-free design. Per-edge DMA descriptors only for gathers; the
dst-side aggregation runs on the Tensor engine via one-hot matmuls that
accumulate straight into PSUM, so the Q7 SWDGE engine (the measured
bottleneck: ~7-8 ns per descriptor) does half the work of v1.

Math (per layer, from PyG GCNConv with self-loops):
    dis = rsqrt(deg + 1)
    hn  = (x @ W) * dis[:, None]
    out[d] = dis[d] * sum_{e: dst[e]=d} hn[src[e]] + b
where the edge list is augmented with one self-edge per node, which makes
the self-loop term an ordinary edge message.

Layout: nodes sharded into 8 contiguous ranges of S=12544 rows (dst
ownership).  Message tables are bf16 with 256-byte gather elements that
pack 2 (layer 1, 64 feats) or 8 (layer 2, 16 feats) rows; a base offset
per subgroup (src&1 / src&7) points the element at the wanted row.  Edges
are sorted by (dst-tile-group, subgroup, dst-tile) and padded per cell to
a multiple of 128 (uniform across cores -> one SPMD program).  For each
128-edge chunk a bf16 one-hot [edge, dst-slot] is built on the Vector
engine (is_equal vs an iota constant; pad edges carry dstl=-1 so their
one-hot row is zero) and a Tensor-engine matmul accumulates the chunk
into the dst tile's PSUM accumulator.  Layer epilogues (relu, W2, dis
scaling) are fused right after each dst tile finishes.  hn2/z2 are
AllGathered; the 1M final edge dot-products reuse the v1 subgroup
machinery unchanged.
"""

import math
import sys

import numpy as np

for _p in ("/opt/trn_rl_repo",):
    if _p not in sys.path:
        sys.path.append(_p)

import concourse.bacc as bacc
import concourse.bass as bass
import concourse.mybir as mybir
import concourse.tile as tile
from concourse import bass_utils
from concourse.masks import make_identity

F32 = mybir.dt.float32
BF16 = mybir.dt.bfloat16
I16 = mybir.dt.int16
AF = mybir.ActivationFunctionType
ALU = mybir.AluOpType


def default_cfg():
    return dict(
        N=100000,
        E=3200000,
        PAIRS=1000000,
        FEAT=128,
        HID=64,
        OUT=16,
        C=8,
        GB=4,  # dst tiles per gather group
        GCAP=6400,  # max edges per dma_gather instruction
        TILE_F=7936,  # pairs per final gather instruction (62*128)
        XT_BLK=8,  # node tiles per xT DMA in the dense phase
        DMA_SCRATCH=16384,
    )


def derive(cfg):
    d = dict(cfg)
    C = d["C"]
    d["S"] = int(math.ceil(d["N"] / C / 128)) * 128  # 12544
    d["NP"] = d["S"] * C  # 100352
    d["G"] = d["NP"] // 128  # 784
    d["GL"] = d["S"] // 128  # 98
    d["NGG"] = (d["GL"] + d["GB"] - 1) // d["GB"]  # 25
    d["M2"] = ((d["N"] - 1) >> 2) + 1  # packed-4 elements in final z2 table
    assert d["M2"] <= 32768
    need = 3 * d["OUT"] + d["M2"] * d["HID"]
    d["NTAB2F"] = max(d["NP"] * d["OUT"], int(math.ceil(need / 2048)) * 2048)
    d["HN1F"] = d["NP"] * d["HID"] + 256  # flat bf16 layer-1 table (+pad)
    d["HN2F"] = d["NP"] * d["OUT"] + 256  # flat bf16 layer-2 table (+pad)
    assert d["G"] % d["XT_BLK"] == 0
    return d


# ---------------------------------------------------------------- host prep


def _wrap16(arr):
    """[..., L] int16 -> [..., 128, L/16] dma_gather idx layout (16-wrap,
    replicated to the 8 Q7 cores)."""
    L = arr.shape[-1]
    lead = arr.shape[:-1]
    a = arr.reshape(lead + (L // 16, 16))
    a = np.moveaxis(a, -1, -2)  # [..., 16, L/16]
    return np.tile(a, (1,) * len(lead) + (8, 1)).astype(np.int16)


def _edge_plan(src, dstl, core_of, nrun, run_of, idx_of, d):
    """Uniform-across-cores padded edge layout for one layer.

    Edges of core c are sorted by (ggrp, run, dst-tile); each
    (ggrp, run, g) cell is padded to a multiple of 128 shared by all
    cores.  Returns:
      gidx  [C, Epad] int16 gather indices (pad = 0)
      dstc  [C, 128, nch] fp32 dst-slot per edge in CONSUMPTION order
            (chunks reordered g-major within each ggrp; pad = -1)
      plan  list over ggrp of dict(runs=[(run, off, npad), ...],
            chunks=[(g, run, tile_col), ...]) with offsets into the
            per-ggrp gather stream
    """
    C, GB, GL, GCAP = d["C"], d["GB"], d["GL"], d["GCAP"]
    NGG = d["NGG"]

    percore = []
    counts = np.zeros((C, NGG, nrun, GB), np.int64)
    for c in range(C):
        m = core_of == c
        s, dl, r = src[m], dstl[m], run_of[m]
        g = dl >> 7
        gg = g >> 2 if GB == 4 else g // GB
        key = ((gg * nrun + r) * GB + (g % GB)).astype(np.int64)
        order = np.argsort(key, kind="stable")
        percore.append((s[order], dl[order], key[order]))
        ks = key[order]
        bounds = np.searchsorted(ks, np.arange(NGG * nrun * GB + 1))
        cnt = (bounds[1:] - bounds[:-1]).reshape(NGG, nrun, GB)
        counts[c] = cnt

    pad = np.maximum(128, ((counts.max(axis=0) + 127) // 128) * 128)  # [NGG,nrun,GB]

    # per-ggrp run offsets and consumption chunk list (uniform)
    plan = []
    total = 0
    nch = 0
    for gg in range(NGG):
        g_lo = gg * GB
        g_hi = min(g_lo + GB, GL)
        runs = []
        off = total
        roff = {}
        for r in range(nrun):
            n = int(pad[gg, r, : g_hi - g_lo].sum())
            roff[r] = total - off
            runs.append((r, total - off, n))
            total += n
        chunks = []
        for gi in range(g_hi - g_lo):
            for r in range(nrun):
                base = roff[r] + int(pad[gg, r, :gi].sum())
                for t in range(int(pad[gg, r, gi]) // 128):
                    chunks.append((g_lo + gi, r, base // 128 + t))
                    nch += 1
        plan.append(dict(goff=off, runs=runs, roff=roff, chunks=chunks,
                         g_lo=g_lo, g_hi=g_hi))
    Epad = total

    gidx = np.zeros((C, Epad), np.int16)
    dstc = np.full((C, 128, nch), -1.0, np.float32)
    for c in range(C):
        s_s, dl_s, ks = percore[c]
        bounds = np.searchsorted(ks, np.arange(NGG * nrun * GB + 1))
        ci = 0
        for gg in range(NGG):
            pgrp = plan[gg]
            for gi in range(pgrp["g_hi"] - pgrp["g_lo"]):
                for r in range(nrun):
                    cell = (gg * nrun + r) * GB + gi
                    b0, b1 = int(bounds[cell]), int(bounds[cell + 1])
                    npad_cell = int(pad[gg, r, gi])
                    base = pgrp["goff"] + pgrp["roff"][r] + \
                        int(pad[gg, r, :gi].sum())
                    gidx[c, base:base + (b1 - b0)] = idx_of(s_s[b0:b1], r)
                    nch_cell = npad_cell // 128
                    vals = np.full(npad_cell, -1.0, np.float32)
                    vals[: b1 - b0] = (dl_s[b0:b1] & 127).astype(np.float32)
                    dstc[c, :, ci:ci + nch_cell] = vals.reshape(nch_cell, 128).T
                    ci += nch_cell
    # gather instruction splits per (ggrp, run), capped at GCAP
    for pgrp in plan:
        pieces = []
        for r, off, n in pgrp["runs"]:
            p = 0
            while p < n:
                t = min(GCAP, n - p)
                pieces.append((r, off + p, t))
                p += t
        pgrp["pieces"] = pieces
    return gidx, dstc, plan, Epad, nch


def prep_host(inputs, cfg):
    d = cfg
    N, C, S, NP = d["N"], d["C"], d["S"], d["NP"]
    FEAT, HID, OUT = d["FEAT"], d["HID"], d["OUT"]
    TILE_F = d["TILE_F"]

    x = np.asarray(inputs["x"], np.float32)
    ei = np.asarray(inputs["edge_index"], np.int64)
    pe = np.asarray(inputs["pos_edge_index"], np.int64)
    ne = np.asarray(inputs["neg_edge_index"], np.int64)
    W1 = np.asarray(inputs["W1"], np.float32)
    b1 = np.asarray(inputs["b1"], np.float32)
    W2 = np.asarray(inputs["W2"], np.float32)
    b2 = np.asarray(inputs["b2"], np.float32)

    src, dst = ei[0], ei[1]

    # self-loop edges make the h*dis^2 term an ordinary message
    ids = np.arange(N, dtype=np.int64)
    asrc = np.concatenate([src, ids])
    adst = np.concatenate([dst, ids])

    import ml_dtypes
    xp = np.zeros((NP, FEAT), np.float32)
    xp[:N] = x
    xT = np.ascontiguousarray(xp.T).astype(ml_dtypes.bfloat16)

    deg = np.bincount(dst, minlength=NP).astype(np.float32) + 1.0
    degp_g = np.ascontiguousarray(deg.reshape(d["G"], 128).T)

    core_of = adst // S
    dstl = adst - core_of * S

    # layer 1: runs by (src>>16, src&1); idx = (src>>1) & 32767
    g1, dc1, plan1, E1, nch1 = _edge_plan(
        asrc, dstl, core_of, 4,
        ((asrc >> 16) * 2 + (asrc & 1)).astype(np.int64),
        lambda sv, r: ((sv >> 1) & 32767).astype(np.int16), d,
    )
    # layer 2: runs by src&7; idx = src>>3
    g2, dc2, plan2, E2, nch2 = _edge_plan(
        asrc, dstl, core_of, 8,
        (asrc & 7).astype(np.int64),
        lambda sv, r: (sv >> 3).astype(np.int16), d,
    )
    gidx1 = _wrap16(g1)  # [C, 128, E1/16]
    gidx2 = _wrap16(g2)

    # ---- final pairs (identical to v1)
    pq = np.concatenate([pe, ne], axis=1)
    P = pq.shape[1]
    PC = P // C
    a = pq[0].reshape(C, PC)
    b = pq[1].reshape(C, PC)
    fkey = (a & 3) * 4 + (b & 3)
    forder = np.argsort(fkey, axis=1, kind="stable")
    fks = np.take_along_axis(fkey, forder, axis=1)
    a_s = np.take_along_axis(a, forder, axis=1)
    b_s = np.take_along_axis(b, forder, axis=1)
    fbounds = np.stack(
        [np.searchsorted(fks[c], np.arange(17)) for c in range(C)]
    )
    fcounts = fbounds[:, 1:] - fbounds[:, :-1]
    n_ft = max(1, int(math.ceil(fcounts.max() / TILE_F)))
    F_sub = n_ft * TILE_F

    fA = np.empty((C, 16, F_sub), np.int16)
    fB = np.empty((C, 16, F_sub), np.int16)
    TJ = TILE_F // 128
    i = np.arange(F_sub)
    t_i = i // TILE_F
    r = i % TILE_F
    lin_i = t_i * TILE_F + (r % 128) * TJ + (r // 128)
    out_pos = np.empty((C, 16 * F_sub), np.int64)
    out_src = np.empty((C, 16 * F_sub), np.int64)
    for c in range(C):
        for s in range(16):
            b0, b1_ = fbounds[c, s], fbounds[c, s + 1]
            cnt = b1_ - b0
            padv = np.arange(F_sub - cnt, dtype=np.int64) % 128
            fA[c, s, :cnt] = a_s[c, b0:b1_] >> 2
            fA[c, s, cnt:] = padv
            fB[c, s, :cnt] = b_s[c, b0:b1_] >> 2
            fB[c, s, cnt:] = padv
            base = s * F_sub
            out_pos[c, base:base + F_sub] = s * n_ft * TILE_F + lin_i
            osrc = np.full(F_sub, -1, np.int64)
            osrc[:cnt] = c * PC + forder[c, b0:b1_]
            out_src[c, base:base + F_sub] = osrc
    fidxA = _wrap16(fA.reshape(C, 16 * F_sub)).reshape(C, 128, -1)
    fidxB = _wrap16(fB.reshape(C, 16 * F_sub)).reshape(C, 128, -1)

    iotax = np.tile(np.arange(128, dtype=np.float32)[None, :], (128, 1))

    in_maps = []
    for c in range(C):
        m = dict(
            xT=np.ascontiguousarray(xT),
            degp_g=degp_g,
            degp_l=np.ascontiguousarray(
                deg[c * S:(c + 1) * S].reshape(d["GL"], 128).T),
            w1=W1.astype(xT.dtype),
            w2=W2.astype(xT.dtype),
            b1r=np.ascontiguousarray(np.tile(b1[None, :], (128, 1))),
            b2r=np.ascontiguousarray(np.tile(b2[None, :], (128, 1))),
            gidx1=np.ascontiguousarray(gidx1[c]),
            gidx2=np.ascontiguousarray(gidx2[c]),
            dstc1=np.ascontiguousarray(dc1[c]),
            dstc2=np.ascontiguousarray(dc2[c]),
            fidxA=np.ascontiguousarray(fidxA[c]),
            fidxB=np.ascontiguousarray(fidxB[c]),
            iotax=iotax,
        )
        in_maps.append(m)

    meta = dict(plan1=_plan_key(plan1), plan2=_plan_key(plan2),
                plans=(plan1, plan2),
                E1=E1, E2=E2, nch1=nch1, nch2=nch2,
                n_ft=n_ft, P=P, out_pos=out_pos, out_src=out_src)
    return in_maps, meta


def _plan_key(plan):
    return tuple(
        (p["goff"], tuple(p["runs"]), tuple(p["chunks"]), p["g_lo"], p["g_hi"],
         tuple(p["pieces"]))
        for p in plan
    )


def assemble(out_maps, meta, cfg):
    P = meta["P"]
    logits = np.zeros(P, np.float32)
    for c in range(cfg["C"]):
        lraw = out_maps[c]["lraw"].reshape(-1)
        pos = meta["out_pos"][c]
        srcg = meta["out_src"][c]
        valid = srcg >= 0
        logits[srcg[valid]] = lraw[pos[valid]]
    return logits


# ---------------------------------------------------------------- device build


def build(cfg, meta):
    d = cfg
    C = d["C"]
    FEAT, HID, OUT = d["FEAT"], d["HID"], d["OUT"]
    S, NP, G, GL = d["S"], d["NP"], d["G"], d["GL"]
    TILE_F = d["TILE_F"]
    plan1, plan2 = meta["plans"]
    E1, E2 = meta["E1"], meta["E2"]
    nch1, nch2 = meta["nch1"], meta["nch2"]
    n_ft = meta["n_ft"]
    F_sub = n_ft * TILE_F
    TJ_F = TILE_F // 128
    XB = d["XT_BLK"]

    nc = bacc.Bacc(
        "TRN2",
        target_bir_lowering=False,
        debug=False,
        enable_asserts=False,
        num_devices=C,
        dynamic_dma_scratch_size=d["DMA_SCRATCH"],
    )

    # I/O
    xT = nc.dram_tensor("xT", [128, NP], BF16, kind="ExternalInput")
    degp_g = nc.dram_tensor("degp_g", [128, G], F32, kind="ExternalInput")
    degp_l = nc.dram_tensor("degp_l", [128, GL], F32, kind="ExternalInput")
    w1 = nc.dram_tensor("w1", [FEAT, HID], BF16, kind="ExternalInput")
    w2 = nc.dram_tensor("w2", [HID, OUT], BF16, kind="ExternalInput")
    b1r = nc.dram_tensor("b1r", [128, HID], F32, kind="ExternalInput")
    b2r = nc.dram_tensor("b2r", [128, OUT], F32, kind="ExternalInput")
    gidx1 = nc.dram_tensor("gidx1", [128, E1 // 16], I16, kind="ExternalInput")
    gidx2 = nc.dram_tensor("gidx2", [128, E2 // 16], I16, kind="ExternalInput")
    dstc1 = nc.dram_tensor("dstc1", [128, nch1], F32, kind="ExternalInput")
    dstc2 = nc.dram_tensor("dstc2", [128, nch2], F32, kind="ExternalInput")
    fidxA = nc.dram_tensor("fidxA", [128, F_sub], I16, kind="ExternalInput")
    fidxB = nc.dram_tensor("fidxB", [128, F_sub], I16, kind="ExternalInput")
    iotax = nc.dram_tensor("iotax", [128, 128], F32, kind="ExternalInput")
    lraw = nc.dram_tensor("lraw", [16 * F_sub], F32, kind="ExternalOutput")

    # internal DRAM
    hn1f = nc.dram_tensor("hn1f", [d["HN1F"]], BF16)
    hn2_sh = nc.dram_tensor("hn2_sh", [S * OUT], BF16)
    z2_sh = nc.dram_tensor("z2_sh", [S * OUT], F32)
    hn2f = nc.dram_tensor("hn2f", [d["HN2F"]], BF16, addr_space="Shared")
    z2_t = nc.dram_tensor("z2_t", [d["NTAB2F"]], F32, addr_space="Shared")

    groups = [list(range(C))]

    def l1_view(run):
        c2, s2 = run >> 1, run & 1
        n_el = 32768 if c2 == 0 else (NP // 2 - 32768)
        base = 64 * s2 + c2 * 32768 * 128
        return hn1f.ap()[base:base + n_el * 128].rearrange("(m e) -> m e", e=128)

    def l2_view(run):
        n_el = NP // 8
        base = 16 * run
        return hn2f.ap()[base:base + n_el * 128].rearrange("(m e) -> m e", e=128)

    def ftab_view(par):
        return z2_t.ap()[par * OUT: par * OUT + d["M2"] * HID].rearrange(
            "(m e) -> m e", e=HID)

    with tile.TileContext(nc) as tc:
        with (
            tc.tile_pool(name="persist", bufs=1) as pP,
            tc.tile_pool(name="idx", bufs=10) as pIdx,
        ):
            # ---- persistent small tensors
            w1_sb = pP.tile([FEAT, HID], BF16)
            nc.sync.dma_start(out=w1_sb[:], in_=w1[:, :])
            w2_sb = pP.tile([HID, OUT], BF16)
            nc.sync.dma_start(out=w2_sb[:], in_=w2[:, :])
            b1_sb = pP.tile([128, HID], F32)
            nc.sync.dma_start(out=b1_sb[:], in_=b1r[:, :])
            b2_sb = pP.tile([128, OUT], F32)
            nc.sync.dma_start(out=b2_sb[:], in_=b2r[:, :])
            iota = pP.tile([128, 128], F32)
            nc.sync.dma_start(out=iota[:], in_=iotax[:, :])
            ident = pP.tile([128, 128], F32)
            make_identity(nc, ident[:])

            dg_raw = pP.tile([128, G], F32)
            nc.sync.dma_start(out=dg_raw[:], in_=degp_g[:, :])
            dis_g = pP.tile([128, G], F32)
            nc.vector.reciprocal(dis_g[:], dg_raw[:])
            nc.scalar.activation(dis_g[:], dis_g[:], AF.Sqrt)

            dl_raw = pP.tile([128, GL], F32)
            nc.sync.dma_start(out=dl_raw[:], in_=degp_l[:, :])
            dis_l = pP.tile([128, GL], F32)
            nc.vector.reciprocal(dis_l[:], dl_raw[:])
            nc.scalar.activation(dis_l[:], dis_l[:], AF.Sqrt)

            # ---- zero table tails (gather views may touch them)
            with tc.tile_pool(name="zero", bufs=1) as pZ:
                ZCOLS = 4096
                zsb = pZ.tile([128, ZCOLS], F32)
                nc.vector.memset(zsb[:], 0.0)

                def zero_flat(flat_ap, n_elems, dt):
                    off = 0
                    zv = zsb[:].bitcast(dt) if dt != F32 else zsb[:]
                    percall = 128 * (ZCOLS * 2 if dt == BF16 else ZCOLS)
                    while off < n_elems:
                        f = min(percall, n_elems - off) // 128
                        nc.sync.dma_start(
                            out=flat_ap[off:off + 128 * f].rearrange(
                                "(p f) -> p f", f=f),
                            in_=zv[:, 0:f],
                        )
                        off += 128 * f

                zero_flat(hn1f.ap()[NP * HID:], 256, BF16)
                zero_flat(hn2f.ap()[NP * OUT:], 256, BF16)
                if d["NTAB2F"] > NP * OUT:
                    zero_flat(z2_t.ap()[NP * OUT:], d["NTAB2F"] - NP * OUT, F32)

            # ---- phase A: full hn1 table (bf16, redundant on every core)
            hn1_r = hn1f.ap()[0:NP * HID].rearrange(
                "(g t p e) -> g p t e", t=XB, p=128, e=HID)
            with tc.tile_pool(name="stream", bufs=3) as pS, tc.tile_pool(
                name="psumA", bufs=4, space="PSUM"
            ) as psA:
                for blk in range(G // XB):
                    xt = pS.tile([128, XB * FEAT], BF16, tag="xt")
                    nc.sync.dma_start(
                        out=xt[:], in_=xT[:, blk * XB * FEAT:(blk + 1) * XB * FEAT]
                    )
                    hn_sb = pS.tile([128, XB * HID], BF16, tag="hn")
                    for t in range(XB):
                        ps = psA.tile([128, HID], F32)
                        nc.tensor.matmul(
                            ps[:],
                            lhsT=xt[:, t * 128:(t + 1) * 128],
                            rhs=w1_sb[:],
                            start=True,
                            stop=True,
                        )
                        g = blk * XB + t
                        nc.vector.tensor_scalar_mul(
                            hn_sb[:, t * HID:(t + 1) * HID], ps[:],
                            dis_g[:, g:g + 1]
                        )
                    nc.sync.dma_start(
                        out=hn1_r[blk],
                        in_=hn_sb[:].rearrange("p (t e) -> p t e", e=HID))

            # ---- layer aggregation loops
            def layer_phase(plan, gidx_in, dstc_in, nch, view_of_run, width,
                            epilogue, pMsg, pOh, psAgg):
                dcol = pP.tile([128, nch], F32, tag=f"dcol{width}")
                nc.sync.dma_start(out=dcol[:], in_=dstc_in[:, :])
                ci_base = 0
                for pgrp in plan:
                    goff = pgrp["goff"]
                    mtiles = {}
                    for r, off, n in pgrp["pieces"]:
                        gi = pIdx.tile([128, n // 16], I16, tag="gi")
                        nc.sync.dma_start(
                            out=gi[:],
                            in_=gidx_in[:, (goff + off) // 16:
                                        (goff + off + n) // 16],
                        )
                        msg = pMsg.tile([128, n // 128, 128], BF16, tag="msg")
                        nc.gpsimd.dma_gather(
                            msg[:], view_of_run(r), gi[:], n, n, 128,
                            single_packet=False,
                        )
                        base = off // 128
                        for t in range(n // 128):
                            mtiles[(r, base + t)] = (msg, t)
                    # consumption: g-major chunk list
                    chunks = pgrp["chunks"]
                    # build one-hots in batches of 8
                    ohs = {}
                    for bstart in range(0, len(chunks), 8):
                        bn = min(8, len(chunks) - bstart)
                        oh8 = pOh.tile([128, 8, 128], BF16, tag="oh8")
                        nc.vector.tensor_tensor(
                            out=oh8[:, 0:bn, :],
                            in0=dcol[:, ci_base + bstart:ci_base + bstart + bn,
                                     None].to_broadcast([128, bn, 128]),
                            in1=iota[:, None, :].to_broadcast([128, bn, 128]),
                            op=ALU.is_equal,
                        )
                        for k in range(bn):
                            ohs[bstart + k] = (oh8, k)
                    cur_g = None
                    agg = None
                    for ci, (g, r, tcol) in enumerate(chunks):
                        if g != cur_g:
                            if cur_g is not None:
                                epilogue(cur_g, agg)
                            cur_g = g
                            agg = psAgg.tile([128, width], F32, tag="agg")
                            first = True
                        msg, t = mtiles[(r, tcol)]
                        oh8, k = ohs[ci]
                        last = (ci + 1 == len(chunks)) or chunks[ci + 1][0] != g
                        nc.tensor.matmul(
                            agg[:],
                            lhsT=oh8[:, k, :],
                            rhs=msg[:, t, 0:width],
                            start=first,
                            stop=last,
                        )
                        first = False
                    epilogue(cur_g, agg)
                    ci_base += len(chunks)

            # ---- layer 1 epilogue: z=relu(agg*dis+b1); hn2=(zT@W2)*dis
            hn2_r = hn2_sh.ap().rearrange("(g p e) -> g p e", p=128, e=OUT)

            with (
                tc.tile_pool(name="msg1", bufs=7) as pMsg,
                tc.tile_pool(name="oh1", bufs=4) as pOh,
                tc.tile_pool(name="agg1", bufs=3, space="PSUM") as psAgg,
                tc.tile_pool(name="epi1", bufs=3) as pEpi,
                tc.tile_pool(name="psepi1", bufs=2, space="PSUM") as psEpi,
            ):
                def epi1(g, agg):
                    zt = pEpi.tile([128, HID], F32, tag="zt")
                    nc.vector.tensor_scalar_mul(zt[:], agg[:], dis_l[:, g:g + 1])
                    nc.vector.tensor_tensor(out=zt[:], in0=zt[:], in1=b1_sb[:],
                                            op=ALU.add)
                    nc.scalar.activation(zt[:], zt[:], AF.Relu)
                    ps_zT = psEpi.tile([64, 128], F32, tag="pszt")
                    nc.tensor.transpose(ps_zT[:], zt[:], ident[:])
                    zT_sb = pEpi.tile([64, 128], BF16, tag="ztT")
                    nc.vector.tensor_copy(zT_sb[:], ps_zT[:])
                    ps_h2 = psEpi.tile([128, OUT], F32, tag="psh2")
                    nc.tensor.matmul(ps_h2[:], lhsT=zT_sb[:], rhs=w2_sb[:],
                                     start=True, stop=True)
                    hn2 = pEpi.tile([128, OUT], BF16, tag="hn2")
                    nc.vector.tensor_scalar_mul(hn2[:], ps_h2[:],
                                                dis_l[:, g:g + 1])
                    nc.sync.dma_start(out=hn2_r[g], in_=hn2[:])

                layer_phase(plan1, gidx1, dstc1, nch1, l1_view, HID, epi1,
                            pMsg, pOh, psAgg)

            nc.gpsimd.collective_compute(
                "AllGather",
                ALU.bypass,
                replica_groups=groups,
                ins=[hn2_sh.ap()],
                outs=[hn2f.ap()[0:NP * OUT]],
            )

            # ---- layer 2 epilogue: z2 = agg2*dis + b2
            z2_r = z2_sh.ap().rearrange("(g p e) -> g p e", p=128, e=OUT)

            with (
                tc.tile_pool(name="msg2", bufs=11) as pMsg2,
                tc.tile_pool(name="oh2", bufs=4) as pOh2,
                tc.tile_pool(name="agg2", bufs=3, space="PSUM") as psAgg2,
                tc.tile_pool(name="epi2", bufs=3) as pEpi2,
            ):
                def epi2(g, agg):
                    z2 = pEpi2.tile([128, OUT], F32, tag="z2")
                    nc.vector.tensor_scalar_mul(z2[:], agg[:], dis_l[:, g:g + 1])
                    nc.vector.tensor_tensor(out=z2[:], in0=z2[:], in1=b2_sb[:],
                                            op=ALU.add)
                    nc.sync.dma_start(out=z2_r[g], in_=z2[:])

                layer_phase(plan2, gidx2, dstc2, nch2, l2_view, OUT, epi2,
                            pMsg2, pOh2, psAgg2)

            nc.gpsimd.collective_compute(
                "AllGather",
                ALU.bypass,
                replica_groups=groups,
                ins=[z2_sh.ap()],
                outs=[z2_t.ap()[0:NP * OUT]],
            )

            # ---- final: edge logits (v1 machinery)
            with tc.tile_pool(name="fin", bufs=3) as pFin:
                colsF = TILE_F // 16
                for s in range(16):
                    for t in range(n_ft):
                        off16 = (s * n_ft + t) * colsF
                        fa = pIdx.tile([128, colsF], I16, tag="fa")
                        nc.sync.dma_start(
                            out=fa[:], in_=fidxA[:, off16:off16 + colsF])
                        fb = pIdx.tile([128, colsF], I16, tag="fb")
                        nc.sync.dma_start(
                            out=fb[:], in_=fidxB[:, off16:off16 + colsF])
                        ma = pFin.tile([128, TJ_F, HID], F32, tag="ma")
                        nc.gpsimd.dma_gather(
                            ma[:], ftab_view(s >> 2), fa[:], TILE_F, TILE_F,
                            HID, single_packet=False,
                        )
                        mb = pFin.tile([128, TJ_F, HID], F32, tag="mb")
                        nc.gpsimd.dma_gather(
                            mb[:], ftab_view(s & 3), fb[:], TILE_F, TILE_F,
                            HID, single_packet=False,
                        )
                        prod = pFin.tile([128, TJ_F, OUT], F32, tag="prod")
                        nc.vector.tensor_tensor(
                            out=prod[:],
                            in0=ma[:, :, 0:OUT],
                            in1=mb[:, :, 0:OUT],
                            op=ALU.mult,
                        )
                        red = pFin.tile([128, TJ_F], F32, tag="red")
                        nc.vector.reduce_sum(
                            out=red[:, :, None],
                            in_=prod[:],
                            axis=mybir.AxisListType.X,
                        )
                        blk = s * n_ft + t
                        nc.sync.dma_start(
                            out=lraw.ap()[blk * TILE_F:(blk + 1) * TILE_F]
                            .rearrange("(p j) -> p j", j=TJ_F),
                            in_=red[:],
                        )

    nc.compile()
    return nc


# ---------------------------------------------------------------- entry point

_CACHE = {}
TRACE = False
LAST = {}


def kernel(**inputs):
    cfg = derive(default_cfg())
    in_maps, meta = prep_host(inputs, cfg)
    key = (meta["plan1"], meta["plan2"], meta["n_ft"])
    if key not in _CACHE:
        _CACHE[key] = build(cfg, meta)
    nc = _CACHE[key]
    res = bass_utils.run_bass_kernel_spmd(
        nc, in_maps, core_ids=list(range(cfg["C"])), trace=TRACE
    )
    LAST["res"] = res
    return assemble(res.results, meta, cfg)


# revision 16
# speedup vs baseline: 1.1656x; 1.1656x over previous
"""2-layer GCN + edge-logit decoder on 8 Trainium2 NeuronCores.

v2: scatter-free design. Per-edge DMA descriptors only for gathers; the
dst-side aggregation runs on the Tensor engine via one-hot matmuls that
accumulate straight into PSUM, so the Q7 SWDGE engine (the measured
bottleneck: ~7-8 ns per descriptor) does half the work of v1.

Math (per layer, from PyG GCNConv with self-loops):
    dis = rsqrt(deg + 1)
    hn  = (x @ W) * dis[:, None]
    out[d] = dis[d] * sum_{e: dst[e]=d} hn[src[e]] + b
where the edge list is augmented with one self-edge per node, which makes
the self-loop term an ordinary edge message.

Layout: nodes sharded into 8 contiguous ranges of S=12544 rows (dst
ownership).  Message tables are bf16 with 256-byte gather elements that
pack 2 (layer 1, 64 feats) or 8 (layer 2, 16 feats) rows; a base offset
per subgroup (src&1 / src&7) points the element at the wanted row.  Edges
are sorted by (dst-tile-group, subgroup, dst-tile) and padded per cell to
a multiple of 128 (uniform across cores -> one SPMD program).  For each
128-edge chunk a bf16 one-hot [edge, dst-slot] is built on the Vector
engine (is_equal vs an iota constant; pad edges carry dstl=-1 so their
one-hot row is zero) and a Tensor-engine matmul accumulates the chunk
into the dst tile's PSUM accumulator.  Layer epilogues (relu, W2, dis
scaling) are fused right after each dst tile finishes.  hn2/z2 are
AllGathered; the 1M final edge dot-products reuse the v1 subgroup
machinery unchanged.
"""

import math
import sys

import numpy as np

for _p in ("/opt/trn_rl_repo",):
    if _p not in sys.path:
        sys.path.append(_p)

import concourse.bacc as bacc
import concourse.bass as bass
import concourse.mybir as mybir
import concourse.tile as tile
from concourse import bass_utils
from concourse.masks import make_identity

F32 = mybir.dt.float32
BF16 = mybir.dt.bfloat16
I16 = mybir.dt.int16
AF = mybir.ActivationFunctionType
ALU = mybir.AluOpType


def default_cfg():
    return dict(
        N=100000,
        E=3200000,
        PAIRS=1000000,
        FEAT=128,
        HID=64,
        OUT=16,
        C=8,
        GB=4,  # dst tiles per gather group
        GCAP=6400,  # max edges per dma_gather instruction
        TILE_F=1024,  # pairs per final gather instruction
        XT_BLK=8,  # node tiles per xT DMA in the dense phase
        DMA_SCRATCH=16384,
    )


def derive(cfg):
    d = dict(cfg)
    C = d["C"]
    d["S"] = int(math.ceil(d["N"] / C / 128)) * 128  # 12544
    d["NP"] = d["S"] * C  # 100352
    d["G"] = d["NP"] // 128  # 784
    d["GL"] = d["S"] // 128  # 98
    d["NGG"] = (d["GL"] + d["GB"] - 1) // d["GB"]  # 25
    d["M2"] = ((d["N"] - 1) >> 2) + 1  # packed-4 elements in final z2 table
    assert d["M2"] <= 32768
    need = 3 * d["OUT"] + d["M2"] * d["HID"]
    d["NTAB2F"] = max(d["NP"] * d["OUT"], int(math.ceil(need / 2048)) * 2048)
    d["HN1F"] = d["NP"] * d["HID"] + 256  # flat bf16 layer-1 table (+pad)
    d["HN2F"] = d["NP"] * d["OUT"] + 256  # flat bf16 layer-2 table (+pad)
    assert d["G"] % d["XT_BLK"] == 0
    return d


# ---------------------------------------------------------------- host prep


def _wrap16(arr):
    """[..., L] int16 -> [..., 128, L/16] dma_gather idx layout (16-wrap,
    replicated to the 8 Q7 cores)."""
    L = arr.shape[-1]
    lead = arr.shape[:-1]
    a = arr.reshape(lead + (L // 16, 16))
    a = np.moveaxis(a, -1, -2)  # [..., 16, L/16]
    return np.tile(a, (1,) * len(lead) + (8, 1)).astype(np.int16)


def _edge_plan(src, dstl, core_of, nrun, run_of, idx_of, d):
    """Uniform-across-cores padded edge layout for one layer.

    Edges of core c are sorted by (ggrp, run, dst-tile); each
    (ggrp, run, g) cell is padded to a multiple of 128 shared by all
    cores.  Returns:
      gidx  [C, Epad] int16 gather indices (pad = 0)
      dstc  [C, 128, nch] fp32 dst-slot per edge in CONSUMPTION order
            (chunks reordered g-major within each ggrp; pad = -1)
      plan  list over ggrp of dict(runs=[(run, off, npad), ...],
            chunks=[(g, run, tile_col), ...]) with offsets into the
            per-ggrp gather stream
    """
    C, GB, GL, GCAP = d["C"], d["GB"], d["GL"], d["GCAP"]
    NGG = d["NGG"]

    percore = []
    counts = np.zeros((C, NGG, nrun, GB), np.int64)
    for c in range(C):
        m = core_of == c
        s, dl, r = src[m], dstl[m], run_of[m]
        g = dl >> 7
        gg = g >> 2 if GB == 4 else g // GB
        key = ((gg * nrun + r) * GB + (g % GB)).astype(np.int64)
        order = np.argsort(key, kind="stable")
        percore.append((s[order], dl[order], key[order]))
        ks = key[order]
        bounds = np.searchsorted(ks, np.arange(NGG * nrun * GB + 1))
        cnt = (bounds[1:] - bounds[:-1]).reshape(NGG, nrun, GB)
        counts[c] = cnt

    pad = np.maximum(128, ((counts.max(axis=0) + 127) // 128) * 128)  # [NGG,nrun,GB]

    # per-ggrp run offsets and consumption chunk list (uniform)
    plan = []
    total = 0
    nch = 0
    for gg in range(NGG):
        g_lo = gg * GB
        g_hi = min(g_lo + GB, GL)
        runs = []
        off = total
        roff = {}
        for r in range(nrun):
            n = int(pad[gg, r, : g_hi - g_lo].sum())
            roff[r] = total - off
            runs.append((r, total - off, n))
            total += n
        chunks = []
        for gi in range(g_hi - g_lo):
            for r in range(nrun):
                base = roff[r] + int(pad[gg, r, :gi].sum())
                for t in range(int(pad[gg, r, gi]) // 128):
                    chunks.append((g_lo + gi, r, base // 128 + t))
                    nch += 1
        plan.append(dict(goff=off, runs=runs, roff=roff, chunks=chunks,
                         g_lo=g_lo, g_hi=g_hi))
    Epad = total

    gidx = np.zeros((C, Epad), np.int16)
    dstc = np.full((C, 128, nch), -1.0, np.float32)
    for c in range(C):
        s_s, dl_s, ks = percore[c]
        bounds = np.searchsorted(ks, np.arange(NGG * nrun * GB + 1))
        ci = 0
        for gg in range(NGG):
            pgrp = plan[gg]
            for gi in range(pgrp["g_hi"] - pgrp["g_lo"]):
                for r in range(nrun):
                    cell = (gg * nrun + r) * GB + gi
                    b0, b1 = int(bounds[cell]), int(bounds[cell + 1])
                    npad_cell = int(pad[gg, r, gi])
                    base = pgrp["goff"] + pgrp["roff"][r] + \
                        int(pad[gg, r, :gi].sum())
                    gidx[c, base:base + (b1 - b0)] = idx_of(s_s[b0:b1], r)
                    nch_cell = npad_cell // 128
                    vals = np.full(npad_cell, -1.0, np.float32)
                    vals[: b1 - b0] = (dl_s[b0:b1] & 127).astype(np.float32)
                    dstc[c, :, ci:ci + nch_cell] = vals.reshape(nch_cell, 128).T
                    ci += nch_cell
    # gather instruction splits per (ggrp, run), capped at GCAP
    for pgrp in plan:
        pieces = []
        for r, off, n in pgrp["runs"]:
            p = 0
            while p < n:
                t = min(GCAP, n - p)
                pieces.append((r, off + p, t))
                p += t
        pgrp["pieces"] = pieces
    return gidx, dstc, plan, Epad, nch


def prep_host(inputs, cfg):
    d = cfg
    N, C, S, NP = d["N"], d["C"], d["S"], d["NP"]
    FEAT, HID, OUT = d["FEAT"], d["HID"], d["OUT"]
    TILE_F = d["TILE_F"]

    x = np.asarray(inputs["x"], np.float32)
    ei = np.asarray(inputs["edge_index"], np.int64)
    pe = np.asarray(inputs["pos_edge_index"], np.int64)
    ne = np.asarray(inputs["neg_edge_index"], np.int64)
    W1 = np.asarray(inputs["W1"], np.float32)
    b1 = np.asarray(inputs["b1"], np.float32)
    W2 = np.asarray(inputs["W2"], np.float32)
    b2 = np.asarray(inputs["b2"], np.float32)

    src, dst = ei[0], ei[1]

    # self-loop edges make the h*dis^2 term an ordinary message
    ids = np.arange(N, dtype=np.int64)
    asrc = np.concatenate([src, ids])
    adst = np.concatenate([dst, ids])

    import ml_dtypes
    xp = np.zeros((NP, FEAT), np.float32)
    xp[:N] = x
    xT = np.ascontiguousarray(xp.T).astype(ml_dtypes.bfloat16)

    deg = np.bincount(dst, minlength=NP).astype(np.float32) + 1.0
    degp_g = np.ascontiguousarray(deg.reshape(d["G"], 128).T)

    core_of = adst // S
    dstl = adst - core_of * S

    # layer 1: runs by (src>>16, src&1); idx = (src>>1) & 32767
    g1, dc1, plan1, E1, nch1 = _edge_plan(
        asrc, dstl, core_of, 4,
        ((asrc >> 16) * 2 + (asrc & 1)).astype(np.int64),
        lambda sv, r: ((sv >> 1) & 32767).astype(np.int16), d,
    )
    # layer 2: runs by src&7; idx = src>>3
    g2, dc2, plan2, E2, nch2 = _edge_plan(
        asrc, dstl, core_of, 8,
        (asrc & 7).astype(np.int64),
        lambda sv, r: (sv >> 3).astype(np.int16), d,
    )
    gidx1 = _wrap16(g1)  # [C, 128, E1/16]
    gidx2 = _wrap16(g2)

    # ---- final pairs (identical to v1)
    pq = np.concatenate([pe, ne], axis=1)
    P = pq.shape[1]
    PC = P // C
    a = pq[0].reshape(C, PC)
    b = pq[1].reshape(C, PC)
    fkey = (a & 3) * 4 + (b & 3)
    forder = np.argsort(fkey, axis=1, kind="stable")
    fks = np.take_along_axis(fkey, forder, axis=1)
    a_s = np.take_along_axis(a, forder, axis=1)
    b_s = np.take_along_axis(b, forder, axis=1)
    fbounds = np.stack(
        [np.searchsorted(fks[c], np.arange(17)) for c in range(C)]
    )
    fcounts = fbounds[:, 1:] - fbounds[:, :-1]
    n_ft = max(1, int(math.ceil(fcounts.max() / TILE_F)))
    F_sub = n_ft * TILE_F

    fA = np.empty((C, 16, F_sub), np.int16)
    fB = np.empty((C, 16, F_sub), np.int16)
    TJ = TILE_F // 128
    i = np.arange(F_sub)
    t_i = i // TILE_F
    r = i % TILE_F
    lin_i = t_i * TILE_F + (r % 128) * TJ + (r // 128)
    out_pos = np.empty((C, 16 * F_sub), np.int64)
    out_src = np.empty((C, 16 * F_sub), np.int64)
    for c in range(C):
        for s in range(16):
            b0, b1_ = fbounds[c, s], fbounds[c, s + 1]
            cnt = b1_ - b0
            padv = np.arange(F_sub - cnt, dtype=np.int64) % 128
            fA[c, s, :cnt] = a_s[c, b0:b1_] >> 2
            fA[c, s, cnt:] = padv
            fB[c, s, :cnt] = b_s[c, b0:b1_] >> 2
            fB[c, s, cnt:] = padv
            base = s * F_sub
            out_pos[c, base:base + F_sub] = s * n_ft * TILE_F + lin_i
            osrc = np.full(F_sub, -1, np.int64)
            osrc[:cnt] = c * PC + forder[c, b0:b1_]
            out_src[c, base:base + F_sub] = osrc
    fidxA = _wrap16(fA.reshape(C, 16 * F_sub)).reshape(C, 128, -1)
    fidxB = _wrap16(fB.reshape(C, 16 * F_sub)).reshape(C, 128, -1)

    iotax = np.tile(np.arange(128, dtype=np.float32)[None, :], (128, 1))

    in_maps = []
    for c in range(C):
        m = dict(
            xT=np.ascontiguousarray(xT),
            degp_g=degp_g,
            degp_l=np.ascontiguousarray(
                deg[c * S:(c + 1) * S].reshape(d["GL"], 128).T),
            w1=W1.astype(xT.dtype),
            w2=W2.astype(xT.dtype),
            b1r=np.ascontiguousarray(np.tile(b1[None, :], (128, 1))),
            b2r=np.ascontiguousarray(np.tile(b2[None, :], (128, 1))),
            gidx1=np.ascontiguousarray(gidx1[c]),
            gidx2=np.ascontiguousarray(gidx2[c]),
            dstc1=np.ascontiguousarray(dc1[c]),
            dstc2=np.ascontiguousarray(dc2[c]),
            fidxA=np.ascontiguousarray(fidxA[c]),
            fidxB=np.ascontiguousarray(fidxB[c]),
            iotax=iotax,
        )
        in_maps.append(m)

    meta = dict(plan1=_plan_key(plan1), plan2=_plan_key(plan2),
                plans=(plan1, plan2),
                E1=E1, E2=E2, nch1=nch1, nch2=nch2,
                n_ft=n_ft, P=P, out_pos=out_pos, out_src=out_src)
    return in_maps, meta


def _plan_key(plan):
    return tuple(
        (p["goff"], tuple(p["runs"]), tuple(p["chunks"]), p["g_lo"], p["g_hi"],
         tuple(p["pieces"]))
        for p in plan
    )


def assemble(out_maps, meta, cfg):
    P = meta["P"]
    logits = np.zeros(P, np.float32)
    for c in range(cfg["C"]):
        lraw = out_maps[c]["lraw"].reshape(-1)
        pos = meta["out_pos"][c]
        srcg = meta["out_src"][c]
        valid = srcg >= 0
        logits[srcg[valid]] = lraw[pos[valid]]
    return logits


# ---------------------------------------------------------------- device build


def build(cfg, meta):
    d = cfg
    C = d["C"]
    FEAT, HID, OUT = d["FEAT"], d["HID"], d["OUT"]
    S, NP, G, GL = d["S"], d["NP"], d["G"], d["GL"]
    TILE_F = d["TILE_F"]
    plan1, plan2 = meta["plans"]
    E1, E2 = meta["E1"], meta["E2"]
    nch1, nch2 = meta["nch1"], meta["nch2"]
    n_ft = meta["n_ft"]
    F_sub = n_ft * TILE_F
    TJ_F = TILE_F // 128
    XB = d["XT_BLK"]

    nc = bacc.Bacc(
        "TRN2",
        target_bir_lowering=False,
        debug=False,
        enable_asserts=False,
        num_devices=C,
        dynamic_dma_scratch_size=d["DMA_SCRATCH"],
    )

    # I/O
    xT = nc.dram_tensor("xT", [128, NP], BF16, kind="ExternalInput")
    degp_g = nc.dram_tensor("degp_g", [128, G], F32, kind="ExternalInput")
    degp_l = nc.dram_tensor("degp_l", [128, GL], F32, kind="ExternalInput")
    w1 = nc.dram_tensor("w1", [FEAT, HID], BF16, kind="ExternalInput")
    w2 = nc.dram_tensor("w2", [HID, OUT], BF16, kind="ExternalInput")
    b1r = nc.dram_tensor("b1r", [128, HID], F32, kind="ExternalInput")
    b2r = nc.dram_tensor("b2r", [128, OUT], F32, kind="ExternalInput")
    gidx1 = nc.dram_tensor("gidx1", [128, E1 // 16], I16, kind="ExternalInput")
    gidx2 = nc.dram_tensor("gidx2", [128, E2 // 16], I16, kind="ExternalInput")
    dstc1 = nc.dram_tensor("dstc1", [128, nch1], F32, kind="ExternalInput")
    dstc2 = nc.dram_tensor("dstc2", [128, nch2], F32, kind="ExternalInput")
    fidxA = nc.dram_tensor("fidxA", [128, F_sub], I16, kind="ExternalInput")
    fidxB = nc.dram_tensor("fidxB", [128, F_sub], I16, kind="ExternalInput")
    iotax = nc.dram_tensor("iotax", [128, 128], F32, kind="ExternalInput")
    lraw = nc.dram_tensor("lraw", [16 * F_sub], F32, kind="ExternalOutput")

    # internal DRAM
    hn1f = nc.dram_tensor("hn1f", [d["HN1F"]], BF16)
    hn2_sh = nc.dram_tensor("hn2_sh", [S * OUT], BF16)
    z2_sh = nc.dram_tensor("z2_sh", [S * OUT], F32)
    hn2f = nc.dram_tensor("hn2f", [d["HN2F"]], BF16, addr_space="Shared")
    z2_t = nc.dram_tensor("z2_t", [d["NTAB2F"]], F32, addr_space="Shared")

    groups = [list(range(C))]

    def l1_view(run):
        c2, s2 = run >> 1, run & 1
        n_el = 32768 if c2 == 0 else (NP // 2 - 32768)
        base = 64 * s2 + c2 * 32768 * 128
        return hn1f.ap()[base:base + n_el * 128].rearrange("(m e) -> m e", e=128)

    def l2_view(run):
        n_el = NP // 8
        base = 16 * run
        return hn2f.ap()[base:base + n_el * 128].rearrange("(m e) -> m e", e=128)

    def ftab_view(par):
        return z2_t.ap()[par * OUT: par * OUT + d["M2"] * HID].rearrange(
            "(m e) -> m e", e=HID)

    with tile.TileContext(nc) as tc:
        with (
            tc.tile_pool(name="persist", bufs=1) as pP,
            tc.tile_pool(name="idx", bufs=10) as pIdx,
        ):
            # ---- persistent small tensors
            w1_sb = pP.tile([FEAT, HID], BF16)
            nc.sync.dma_start(out=w1_sb[:], in_=w1[:, :])
            w2_sb = pP.tile([HID, OUT], BF16)
            nc.sync.dma_start(out=w2_sb[:], in_=w2[:, :])
            b1_sb = pP.tile([128, HID], F32)
            nc.sync.dma_start(out=b1_sb[:], in_=b1r[:, :])
            b2_sb = pP.tile([128, OUT], F32)
            nc.sync.dma_start(out=b2_sb[:], in_=b2r[:, :])
            iota = pP.tile([128, 128], F32)
            nc.sync.dma_start(out=iota[:], in_=iotax[:, :])
            ident = pP.tile([128, 128], F32)
            make_identity(nc, ident[:])

            dg_raw = pP.tile([128, G], F32)
            nc.sync.dma_start(out=dg_raw[:], in_=degp_g[:, :])
            dis_g = pP.tile([128, G], F32)
            nc.vector.reciprocal(dis_g[:], dg_raw[:])
            nc.scalar.activation(dis_g[:], dis_g[:], AF.Sqrt)

            dl_raw = pP.tile([128, GL], F32)
            nc.sync.dma_start(out=dl_raw[:], in_=degp_l[:, :])
            dis_l = pP.tile([128, GL], F32)
            nc.vector.reciprocal(dis_l[:], dl_raw[:])
            nc.scalar.activation(dis_l[:], dis_l[:], AF.Sqrt)

            # ---- zero table tails (gather views may touch them)
            with tc.tile_pool(name="zero", bufs=1) as pZ:
                ZCOLS = 4096
                zsb = pZ.tile([128, ZCOLS], F32)
                nc.vector.memset(zsb[:], 0.0)

                def zero_flat(flat_ap, n_elems, dt):
                    off = 0
                    zv = zsb[:].bitcast(dt) if dt != F32 else zsb[:]
                    percall = 128 * (ZCOLS * 2 if dt == BF16 else ZCOLS)
                    while off < n_elems:
                        f = min(percall, n_elems - off) // 128
                        nc.sync.dma_start(
                            out=flat_ap[off:off + 128 * f].rearrange(
                                "(p f) -> p f", f=f),
                            in_=zv[:, 0:f],
                        )
                        off += 128 * f

                zero_flat(hn1f.ap()[NP * HID:], 256, BF16)
                zero_flat(hn2f.ap()[NP * OUT:], 256, BF16)
                if d["NTAB2F"] > NP * OUT:
                    zero_flat(z2_t.ap()[NP * OUT:], d["NTAB2F"] - NP * OUT, F32)

            # ---- phase A: full hn1 table (bf16, redundant on every core)
            hn1_r = hn1f.ap()[0:NP * HID].rearrange(
                "(g t p e) -> g p t e", t=XB, p=128, e=HID)
            with tc.tile_pool(name="stream", bufs=3) as pS, tc.tile_pool(
                name="psumA", bufs=4, space="PSUM"
            ) as psA:
                for blk in range(G // XB):
                    xt = pS.tile([128, XB * FEAT], BF16, tag="xt")
                    nc.sync.dma_start(
                        out=xt[:], in_=xT[:, blk * XB * FEAT:(blk + 1) * XB * FEAT]
                    )
                    hn_sb = pS.tile([128, XB * HID], BF16, tag="hn")
                    for t in range(XB):
                        ps = psA.tile([128, HID], F32)
                        nc.tensor.matmul(
                            ps[:],
                            lhsT=xt[:, t * 128:(t + 1) * 128],
                            rhs=w1_sb[:],
                            start=True,
                            stop=True,
                        )
                        g = blk * XB + t
                        nc.vector.tensor_scalar_mul(
                            hn_sb[:, t * HID:(t + 1) * HID], ps[:],
                            dis_g[:, g:g + 1]
                        )
                    nc.sync.dma_start(
                        out=hn1_r[blk],
                        in_=hn_sb[:].rearrange("p (t e) -> p t e", e=HID))

            # ---- layer aggregation loops
            def layer_phase(plan, gidx_in, dstc_in, nch, view_of_run, width,
                            epilogue, pMsg, pOh, psAgg):
                dcol = pP.tile([128, nch], F32, tag=f"dcol{width}")
                nc.sync.dma_start(out=dcol[:], in_=dstc_in[:, :])
                ci_base = 0
                for pgrp in plan:
                    goff = pgrp["goff"]
                    mtiles = {}
                    for r, off, n in pgrp["pieces"]:
                        gi = pIdx.tile([128, n // 16], I16, tag="gi")
                        nc.sync.dma_start(
                            out=gi[:],
                            in_=gidx_in[:, (goff + off) // 16:
                                        (goff + off + n) // 16],
                        )
                        msg = pMsg.tile([128, n // 128, 128], BF16, tag="msg")
                        nc.gpsimd.dma_gather(
                            msg[:], view_of_run(r), gi[:], n, n, 128,
                            single_packet=False,
                        )
                        base = off // 128
                        for t in range(n // 128):
                            mtiles[(r, base + t)] = (msg, t)
                    # consumption: g-major chunk list
                    chunks = pgrp["chunks"]
                    # build one-hots in batches of 8
                    ohs = {}
                    for bstart in range(0, len(chunks), 8):
                        bn = min(8, len(chunks) - bstart)
                        oh8 = pOh.tile([128, 8, 128], BF16, tag="oh8")
                        nc.vector.tensor_tensor(
                            out=oh8[:, 0:bn, :],
                            in0=dcol[:, ci_base + bstart:ci_base + bstart + bn,
                                     None].to_broadcast([128, bn, 128]),
                            in1=iota[:, None, :].to_broadcast([128, bn, 128]),
                            op=ALU.is_equal,
                        )
                        for k in range(bn):
                            ohs[bstart + k] = (oh8, k)
                    cur_g = None
                    agg = None
                    for ci, (g, r, tcol) in enumerate(chunks):
                        if g != cur_g:
                            if cur_g is not None:
                                epilogue(cur_g, agg)
                            cur_g = g
                            agg = psAgg.tile([128, width], F32, tag="agg")
                            first = True
                        msg, t = mtiles[(r, tcol)]
                        oh8, k = ohs[ci]
                        last = (ci + 1 == len(chunks)) or chunks[ci + 1][0] != g
                        nc.tensor.matmul(
                            agg[:],
                            lhsT=oh8[:, k, :],
                            rhs=msg[:, t, 0:width],
                            start=first,
                            stop=last,
                        )
                        first = False
                    epilogue(cur_g, agg)
                    ci_base += len(chunks)

            # ---- layer 1 epilogue: z=relu(agg*dis+b1); hn2=(zT@W2)*dis
            hn2_r = hn2_sh.ap().rearrange("(g p e) -> g p e", p=128, e=OUT)

            with (
                tc.tile_pool(name="msg1", bufs=7) as pMsg,
                tc.tile_pool(name="oh1", bufs=4) as pOh,
                tc.tile_pool(name="agg1", bufs=3, space="PSUM") as psAgg,
                tc.tile_pool(name="epi1", bufs=3) as pEpi,
                tc.tile_pool(name="psepi1", bufs=2, space="PSUM") as psEpi,
            ):
                def epi1(g, agg):
                    zt = pEpi.tile([128, HID], F32, tag="zt")
                    nc.vector.tensor_scalar_mul(zt[:], agg[:], dis_l[:, g:g + 1])
                    nc.vector.tensor_tensor(out=zt[:], in0=zt[:], in1=b1_sb[:],
                                            op=ALU.add)
                    nc.scalar.activation(zt[:], zt[:], AF.Relu)
                    ps_zT = psEpi.tile([64, 128], F32, tag="pszt")
                    nc.tensor.transpose(ps_zT[:], zt[:], ident[:])
                    zT_sb = pEpi.tile([64, 128], BF16, tag="ztT")
                    nc.vector.tensor_copy(zT_sb[:], ps_zT[:])
                    ps_h2 = psEpi.tile([128, OUT], F32, tag="psh2")
                    nc.tensor.matmul(ps_h2[:], lhsT=zT_sb[:], rhs=w2_sb[:],
                                     start=True, stop=True)
                    hn2 = pEpi.tile([128, OUT], BF16, tag="hn2")
                    nc.vector.tensor_scalar_mul(hn2[:], ps_h2[:],
                                                dis_l[:, g:g + 1])
                    nc.sync.dma_start(out=hn2_r[g], in_=hn2[:])

                layer_phase(plan1, gidx1, dstc1, nch1, l1_view, HID, epi1,
                            pMsg, pOh, psAgg)

            nc.gpsimd.collective_compute(
                "AllGather",
                ALU.bypass,
                replica_groups=groups,
                ins=[hn2_sh.ap()],
                outs=[hn2f.ap()[0:NP * OUT]],
            )

            # ---- layer 2 epilogue: z2 = agg2*dis + b2
            z2_r = z2_sh.ap().rearrange("(g p e) -> g p e", p=128, e=OUT)

            with (
                tc.tile_pool(name="msg2", bufs=11) as pMsg2,
                tc.tile_pool(name="oh2", bufs=4) as pOh2,
                tc.tile_pool(name="agg2", bufs=3, space="PSUM") as psAgg2,
                tc.tile_pool(name="epi2", bufs=3) as pEpi2,
            ):
                def epi2(g, agg):
                    z2 = pEpi2.tile([128, OUT], F32, tag="z2")
                    nc.vector.tensor_scalar_mul(z2[:], agg[:], dis_l[:, g:g + 1])
                    nc.vector.tensor_tensor(out=z2[:], in0=z2[:], in1=b2_sb[:],
                                            op=ALU.add)
                    nc.sync.dma_start(out=z2_r[g], in_=z2[:])

                layer_phase(plan2, gidx2, dstc2, nch2, l2_view, OUT, epi2,
                            pMsg2, pOh2, psAgg2)

            nc.gpsimd.collective_compute(
                "AllGather",
                ALU.bypass,
                replica_groups=groups,
                ins=[z2_sh.ap()],
                outs=[z2_t.ap()[0:NP * OUT]],
            )

            # ---- final: edge logits (v1 machinery)
            with tc.tile_pool(name="fin", bufs=3) as pFin:
                colsF = TILE_F // 16
                for s in range(16):
                    for t in range(n_ft):
                        off16 = (s * n_ft + t) * colsF
                        fa = pIdx.tile([128, colsF], I16, tag="fa")
                        nc.sync.dma_start(
                            out=fa[:], in_=fidxA[:, off16:off16 + colsF])
                        fb = pIdx.tile([128, colsF], I16, tag="fb")
                        nc.sync.dma_start(
                            out=fb[:], in_=fidxB[:, off16:off16 + colsF])
                        ma = pFin.tile([128, TJ_F, HID], F32, tag="ma")
                        nc.gpsimd.dma_gather(
                            ma[:], ftab_view(s >> 2), fa[:], TILE_F, TILE_F,
                            HID, single_packet=False,
                        )
                        mb = pFin.tile([128, TJ_F, HID], F32, tag="mb")
                        nc.gpsimd.dma_gather(
                            mb[:], ftab_view(s & 3), fb[:], TILE_F, TILE_F,
                            HID, single_packet=False,
                        )
                        prod = pFin.tile([128, TJ_F, OUT], F32, tag="prod")
                        nc.vector.tensor_tensor(
                            out=prod[:],
                            in0=ma[:, :, 0:OUT],
                            in1=mb[:, :, 0:OUT],
                            op=ALU.mult,
                        )
                        red = pFin.tile([128, TJ_F], F32, tag="red")
                        nc.vector.reduce_sum(
                            out=red[:, :, None],
                            in_=prod[:],
                            axis=mybir.AxisListType.X,
                        )
                        blk = s * n_ft + t
                        nc.sync.dma_start(
                            out=lraw.ap()[blk * TILE_F:(blk + 1) * TILE_F]
                            .rearrange("(p j) -> p j", j=TJ_F),
                            in_=red[:],
                        )

    nc.compile()
    return nc


# ---------------------------------------------------------------- entry point

_CACHE = {}
TRACE = False
LAST = {}


def kernel(**inputs):
    cfg = derive(default_cfg())
    in_maps, meta = prep_host(inputs, cfg)
    key = (meta["plan1"], meta["plan2"], meta["n_ft"])
    if key not in _CACHE:
        _CACHE[key] = build(cfg, meta)
    nc = _CACHE[key]
    res = bass_utils.run_bass_kernel_spmd(
        nc, in_maps, core_ids=list(range(cfg["C"])), trace=TRACE
    )
    LAST["res"] = res
    return assemble(res.results, meta, cfg)


# revision 18
# speedup vs baseline: 1.1824x; 1.0144x over previous
"""2-layer GCN + edge-logit decoder on 8 Trainium2 NeuronCores.

v2: scatter-free design. Per-edge DMA descriptors only for gathers; the
dst-side aggregation runs on the Tensor engine via one-hot matmuls that
accumulate straight into PSUM, so the Q7 SWDGE engine (the measured
bottleneck: ~7-8 ns per descriptor) does half the work of v1.

Math (per layer, from PyG GCNConv with self-loops):
    dis = rsqrt(deg + 1)
    hn  = (x @ W) * dis[:, None]
    out[d] = dis[d] * sum_{e: dst[e]=d} hn[src[e]] + b
where the edge list is augmented with one self-edge per node, which makes
the self-loop term an ordinary edge message.

Layout: nodes sharded into 8 contiguous ranges of S=12544 rows (dst
ownership).  Message tables are bf16 with 256-byte gather elements that
pack 2 (layer 1, 64 feats) or 8 (layer 2, 16 feats) rows; a base offset
per subgroup (src&1 / src&7) points the element at the wanted row.  Edges
are sorted by (dst-tile-group, subgroup, dst-tile) and padded per cell to
a multiple of 128 (uniform across cores -> one SPMD program).  For each
128-edge chunk a bf16 one-hot [edge, dst-slot] is built on the Vector
engine (is_equal vs an iota constant; pad edges carry dstl=-1 so their
one-hot row is zero) and a Tensor-engine matmul accumulates the chunk
into the dst tile's PSUM accumulator.  Layer epilogues (relu, W2, dis
scaling) are fused right after each dst tile finishes.  hn2/z2 are
AllGathered; the 1M final edge dot-products reuse the v1 subgroup
machinery unchanged.
"""

import math
import sys

import numpy as np

for _p in ("/opt/trn_rl_repo",):
    if _p not in sys.path:
        sys.path.append(_p)

import concourse.bacc as bacc
import concourse.bass as bass
import concourse.mybir as mybir
import concourse.tile as tile
from concourse import bass_utils
from concourse.masks import make_identity

F32 = mybir.dt.float32
BF16 = mybir.dt.bfloat16
I16 = mybir.dt.int16
AF = mybir.ActivationFunctionType
ALU = mybir.AluOpType


def default_cfg():
    return dict(
        N=100000,
        E=3200000,
        PAIRS=1000000,
        FEAT=128,
        HID=64,
        OUT=16,
        C=8,
        GB=4,  # dst tiles per gather group
        GCAP=6400,  # max edges per dma_gather instruction
        TILE_F=4096,  # pairs per final gather instruction
        XT_BLK=8,  # node tiles per xT DMA in the dense phase
        DMA_SCRATCH=32768,
    )


def derive(cfg):
    d = dict(cfg)
    C = d["C"]
    d["S"] = int(math.ceil(d["N"] / C / 128)) * 128  # 12544
    d["NP"] = d["S"] * C  # 100352
    d["G"] = d["NP"] // 128  # 784
    d["GL"] = d["S"] // 128  # 98
    d["NGG"] = (d["GL"] + d["GB"] - 1) // d["GB"]  # 25
    d["M2"] = ((d["N"] - 1) >> 2) + 1  # packed-4 elements in final z2 table
    assert d["M2"] <= 32768
    need = 3 * d["OUT"] + d["M2"] * d["HID"]
    d["NTAB2F"] = max(d["NP"] * d["OUT"], int(math.ceil(need / 2048)) * 2048)
    d["HN1F"] = d["NP"] * d["HID"] + 256  # flat bf16 layer-1 table (+pad)
    d["HN2F"] = d["NP"] * d["OUT"] + 256  # flat bf16 layer-2 table (+pad)
    assert d["G"] % d["XT_BLK"] == 0
    return d


# ---------------------------------------------------------------- host prep


def _wrap16(arr):
    """[..., L] int16 -> [..., 128, L/16] dma_gather idx layout (16-wrap,
    replicated to the 8 Q7 cores)."""
    L = arr.shape[-1]
    lead = arr.shape[:-1]
    a = arr.reshape(lead + (L // 16, 16))
    a = np.moveaxis(a, -1, -2)  # [..., 16, L/16]
    return np.tile(a, (1,) * len(lead) + (8, 1)).astype(np.int16)


def _edge_plan(src, dstl, core_of, nrun, run_of, idx_of, d):
    """Uniform-across-cores padded edge layout for one layer.

    Edges of core c are sorted by (ggrp, run, dst-tile); each
    (ggrp, run, g) cell is padded to a multiple of 128 shared by all
    cores.  Returns:
      gidx  [C, Epad] int16 gather indices (pad = 0)
      dstc  [C, 128, nch] fp32 dst-slot per edge in CONSUMPTION order
            (chunks reordered g-major within each ggrp; pad = -1)
      plan  list over ggrp of dict(runs=[(run, off, npad), ...],
            chunks=[(g, run, tile_col), ...]) with offsets into the
            per-ggrp gather stream
    """
    C, GB, GL, GCAP = d["C"], d["GB"], d["GL"], d["GCAP"]
    NGG = d["NGG"]

    percore = []
    counts = np.zeros((C, NGG, nrun, GB), np.int64)
    for c in range(C):
        m = core_of == c
        s, dl, r = src[m], dstl[m], run_of[m]
        g = dl >> 7
        gg = g >> 2 if GB == 4 else g // GB
        key = ((gg * nrun + r) * GB + (g % GB)).astype(np.int64)
        order = np.argsort(key, kind="stable")
        percore.append((s[order], dl[order], key[order]))
        ks = key[order]
        bounds = np.searchsorted(ks, np.arange(NGG * nrun * GB + 1))
        cnt = (bounds[1:] - bounds[:-1]).reshape(NGG, nrun, GB)
        counts[c] = cnt

    pad = np.maximum(128, ((counts.max(axis=0) + 127) // 128) * 128)  # [NGG,nrun,GB]

    # per-ggrp run offsets and consumption chunk list (uniform)
    plan = []
    total = 0
    nch = 0
    for gg in range(NGG):
        g_lo = gg * GB
        g_hi = min(g_lo + GB, GL)
        runs = []
        off = total
        roff = {}
        for r in range(nrun):
            n = int(pad[gg, r, : g_hi - g_lo].sum())
            roff[r] = total - off
            runs.append((r, total - off, n))
            total += n
        chunks = []
        for gi in range(g_hi - g_lo):
            for r in range(nrun):
                base = roff[r] + int(pad[gg, r, :gi].sum())
                for t in range(int(pad[gg, r, gi]) // 128):
                    chunks.append((g_lo + gi, r, base // 128 + t))
                    nch += 1
        plan.append(dict(goff=off, runs=runs, roff=roff, chunks=chunks,
                         g_lo=g_lo, g_hi=g_hi))
    Epad = total

    gidx = np.zeros((C, Epad), np.int16)
    dstc = np.full((C, 128, nch), -1.0, np.float32)
    for c in range(C):
        s_s, dl_s, ks = percore[c]
        bounds = np.searchsorted(ks, np.arange(NGG * nrun * GB + 1))
        ci = 0
        for gg in range(NGG):
            pgrp = plan[gg]
            for gi in range(pgrp["g_hi"] - pgrp["g_lo"]):
                for r in range(nrun):
                    cell = (gg * nrun + r) * GB + gi
                    b0, b1 = int(bounds[cell]), int(bounds[cell + 1])
                    npad_cell = int(pad[gg, r, gi])
                    base = pgrp["goff"] + pgrp["roff"][r] + \
                        int(pad[gg, r, :gi].sum())
                    gidx[c, base:base + (b1 - b0)] = idx_of(s_s[b0:b1], r)
                    nch_cell = npad_cell // 128
                    vals = np.full(npad_cell, -1.0, np.float32)
                    vals[: b1 - b0] = (dl_s[b0:b1] & 127).astype(np.float32)
                    dstc[c, :, ci:ci + nch_cell] = vals.reshape(nch_cell, 128).T
                    ci += nch_cell
    # gather instruction splits per (ggrp, run), capped at GCAP
    for pgrp in plan:
        pieces = []
        for r, off, n in pgrp["runs"]:
            p = 0
            while p < n:
                t = min(GCAP, n - p)
                pieces.append((r, off + p, t))
                p += t
        pgrp["pieces"] = pieces
    return gidx, dstc, plan, Epad, nch


def prep_host(inputs, cfg):
    d = cfg
    N, C, S, NP = d["N"], d["C"], d["S"], d["NP"]
    FEAT, HID, OUT = d["FEAT"], d["HID"], d["OUT"]
    TILE_F = d["TILE_F"]

    x = np.asarray(inputs["x"], np.float32)
    ei = np.asarray(inputs["edge_index"], np.int64)
    pe = np.asarray(inputs["pos_edge_index"], np.int64)
    ne = np.asarray(inputs["neg_edge_index"], np.int64)
    W1 = np.asarray(inputs["W1"], np.float32)
    b1 = np.asarray(inputs["b1"], np.float32)
    W2 = np.asarray(inputs["W2"], np.float32)
    b2 = np.asarray(inputs["b2"], np.float32)

    src, dst = ei[0], ei[1]

    # self-loop edges make the h*dis^2 term an ordinary message
    ids = np.arange(N, dtype=np.int64)
    asrc = np.concatenate([src, ids])
    adst = np.concatenate([dst, ids])

    import ml_dtypes
    xp = np.zeros((NP, FEAT), np.float32)
    xp[:N] = x
    xT = np.ascontiguousarray(xp.T).astype(ml_dtypes.bfloat16)

    deg = np.bincount(dst, minlength=NP).astype(np.float32) + 1.0
    degp_g = np.ascontiguousarray(deg.reshape(d["G"], 128).T)

    core_of = adst // S
    dstl = adst - core_of * S

    # layer 1: runs by (src>>16, src&1); idx = (src>>1) & 32767
    g1, dc1, plan1, E1, nch1 = _edge_plan(
        asrc, dstl, core_of, 4,
        ((asrc >> 16) * 2 + (asrc & 1)).astype(np.int64),
        lambda sv, r: ((sv >> 1) & 32767).astype(np.int16), d,
    )
    # layer 2: runs by src&7; idx = src>>3
    g2, dc2, plan2, E2, nch2 = _edge_plan(
        asrc, dstl, core_of, 8,
        (asrc & 7).astype(np.int64),
        lambda sv, r: (sv >> 3).astype(np.int16), d,
    )
    gidx1 = _wrap16(g1)  # [C, 128, E1/16]
    gidx2 = _wrap16(g2)

    # ---- final pairs (identical to v1)
    pq = np.concatenate([pe, ne], axis=1)
    P = pq.shape[1]
    PC = P // C
    a = pq[0].reshape(C, PC)
    b = pq[1].reshape(C, PC)
    fkey = (a & 3) * 4 + (b & 3)
    forder = np.argsort(fkey, axis=1, kind="stable")
    fks = np.take_along_axis(fkey, forder, axis=1)
    a_s = np.take_along_axis(a, forder, axis=1)
    b_s = np.take_along_axis(b, forder, axis=1)
    fbounds = np.stack(
        [np.searchsorted(fks[c], np.arange(17)) for c in range(C)]
    )
    fcounts = fbounds[:, 1:] - fbounds[:, :-1]
    n_ft = max(1, int(math.ceil(fcounts.max() / TILE_F)))
    F_sub = n_ft * TILE_F

    fA = np.empty((C, 16, F_sub), np.int16)
    fB = np.empty((C, 16, F_sub), np.int16)
    TJ = TILE_F // 128
    i = np.arange(F_sub)
    t_i = i // TILE_F
    r = i % TILE_F
    lin_i = t_i * TILE_F + (r % 128) * TJ + (r // 128)
    out_pos = np.empty((C, 16 * F_sub), np.int64)
    out_src = np.empty((C, 16 * F_sub), np.int64)
    for c in range(C):
        for s in range(16):
            b0, b1_ = fbounds[c, s], fbounds[c, s + 1]
            cnt = b1_ - b0
            padv = np.arange(F_sub - cnt, dtype=np.int64) % 128
            fA[c, s, :cnt] = a_s[c, b0:b1_] >> 2
            fA[c, s, cnt:] = padv
            fB[c, s, :cnt] = b_s[c, b0:b1_] >> 2
            fB[c, s, cnt:] = padv
            base = s * F_sub
            out_pos[c, base:base + F_sub] = s * n_ft * TILE_F + lin_i
            osrc = np.full(F_sub, -1, np.int64)
            osrc[:cnt] = c * PC + forder[c, b0:b1_]
            out_src[c, base:base + F_sub] = osrc
    fidxA = _wrap16(fA.reshape(C, 16 * F_sub)).reshape(C, 128, -1)
    fidxB = _wrap16(fB.reshape(C, 16 * F_sub)).reshape(C, 128, -1)

    iotax = np.tile(np.arange(128, dtype=np.float32)[None, :], (128, 1))

    in_maps = []
    for c in range(C):
        m = dict(
            xT=np.ascontiguousarray(xT),
            degp_g=degp_g,
            degp_l=np.ascontiguousarray(
                deg[c * S:(c + 1) * S].reshape(d["GL"], 128).T),
            w1=W1.astype(xT.dtype),
            w2=W2.astype(xT.dtype),
            b1r=np.ascontiguousarray(np.tile(b1[None, :], (128, 1))),
            b2r=np.ascontiguousarray(np.tile(b2[None, :], (128, 1))),
            gidx1=np.ascontiguousarray(gidx1[c]),
            gidx2=np.ascontiguousarray(gidx2[c]),
            dstc1=np.ascontiguousarray(dc1[c]),
            dstc2=np.ascontiguousarray(dc2[c]),
            fidxA=np.ascontiguousarray(fidxA[c]),
            fidxB=np.ascontiguousarray(fidxB[c]),
            iotax=iotax,
        )
        in_maps.append(m)

    meta = dict(plan1=_plan_key(plan1), plan2=_plan_key(plan2),
                plans=(plan1, plan2),
                E1=E1, E2=E2, nch1=nch1, nch2=nch2,
                n_ft=n_ft, P=P, out_pos=out_pos, out_src=out_src)
    return in_maps, meta


def _plan_key(plan):
    return tuple(
        (p["goff"], tuple(p["runs"]), tuple(p["chunks"]), p["g_lo"], p["g_hi"],
         tuple(p["pieces"]))
        for p in plan
    )


def assemble(out_maps, meta, cfg):
    P = meta["P"]
    logits = np.zeros(P, np.float32)
    for c in range(cfg["C"]):
        lraw = out_maps[c]["lraw"].reshape(-1)
        pos = meta["out_pos"][c]
        srcg = meta["out_src"][c]
        valid = srcg >= 0
        logits[srcg[valid]] = lraw[pos[valid]]
    return logits


# ---------------------------------------------------------------- device build


def build(cfg, meta):
    d = cfg
    C = d["C"]
    FEAT, HID, OUT = d["FEAT"], d["HID"], d["OUT"]
    S, NP, G, GL = d["S"], d["NP"], d["G"], d["GL"]
    TILE_F = d["TILE_F"]
    plan1, plan2 = meta["plans"]
    E1, E2 = meta["E1"], meta["E2"]
    nch1, nch2 = meta["nch1"], meta["nch2"]
    n_ft = meta["n_ft"]
    F_sub = n_ft * TILE_F
    TJ_F = TILE_F // 128
    XB = d["XT_BLK"]

    nc = bacc.Bacc(
        "TRN2",
        target_bir_lowering=False,
        debug=False,
        enable_asserts=False,
        num_devices=C,
        dynamic_dma_scratch_size=d["DMA_SCRATCH"],
    )

    # I/O
    xT = nc.dram_tensor("xT", [128, NP], BF16, kind="ExternalInput")
    degp_g = nc.dram_tensor("degp_g", [128, G], F32, kind="ExternalInput")
    degp_l = nc.dram_tensor("degp_l", [128, GL], F32, kind="ExternalInput")
    w1 = nc.dram_tensor("w1", [FEAT, HID], BF16, kind="ExternalInput")
    w2 = nc.dram_tensor("w2", [HID, OUT], BF16, kind="ExternalInput")
    b1r = nc.dram_tensor("b1r", [128, HID], F32, kind="ExternalInput")
    b2r = nc.dram_tensor("b2r", [128, OUT], F32, kind="ExternalInput")
    gidx1 = nc.dram_tensor("gidx1", [128, E1 // 16], I16, kind="ExternalInput")
    gidx2 = nc.dram_tensor("gidx2", [128, E2 // 16], I16, kind="ExternalInput")
    dstc1 = nc.dram_tensor("dstc1", [128, nch1], F32, kind="ExternalInput")
    dstc2 = nc.dram_tensor("dstc2", [128, nch2], F32, kind="ExternalInput")
    fidxA = nc.dram_tensor("fidxA", [128, F_sub], I16, kind="ExternalInput")
    fidxB = nc.dram_tensor("fidxB", [128, F_sub], I16, kind="ExternalInput")
    iotax = nc.dram_tensor("iotax", [128, 128], F32, kind="ExternalInput")
    lraw = nc.dram_tensor("lraw", [16 * F_sub], F32, kind="ExternalOutput")

    # internal DRAM
    hn1f = nc.dram_tensor("hn1f", [d["HN1F"]], BF16)
    hn2_sh = nc.dram_tensor("hn2_sh", [S * OUT], BF16)
    z2_sh = nc.dram_tensor("z2_sh", [S * OUT], F32)
    hn2f = nc.dram_tensor("hn2f", [d["HN2F"]], BF16, addr_space="Shared")
    z2_t = nc.dram_tensor("z2_t", [d["NTAB2F"]], F32, addr_space="Shared")

    groups = [list(range(C))]

    def l1_view(run):
        c2, s2 = run >> 1, run & 1
        n_el = 32768 if c2 == 0 else (NP // 2 - 32768)
        base = 64 * s2 + c2 * 32768 * 128
        return hn1f.ap()[base:base + n_el * 128].rearrange("(m e) -> m e", e=128)

    def l2_view(run):
        n_el = NP // 8
        base = 16 * run
        return hn2f.ap()[base:base + n_el * 128].rearrange("(m e) -> m e", e=128)

    def ftab_view(par):
        return z2_t.ap()[par * OUT: par * OUT + d["M2"] * HID].rearrange(
            "(m e) -> m e", e=HID)

    with tile.TileContext(nc) as tc:
        with (
            tc.tile_pool(name="persist", bufs=1) as pP,
            tc.tile_pool(name="idx", bufs=10) as pIdx,
        ):
            # ---- persistent small tensors
            w1_sb = pP.tile([FEAT, HID], BF16)
            nc.sync.dma_start(out=w1_sb[:], in_=w1[:, :])
            w2_sb = pP.tile([HID, OUT], BF16)
            nc.sync.dma_start(out=w2_sb[:], in_=w2[:, :])
            b1_sb = pP.tile([128, HID], F32)
            nc.sync.dma_start(out=b1_sb[:], in_=b1r[:, :])
            b2_sb = pP.tile([128, OUT], F32)
            nc.sync.dma_start(out=b2_sb[:], in_=b2r[:, :])
            iota = pP.tile([128, 128], F32)
            nc.sync.dma_start(out=iota[:], in_=iotax[:, :])
            ident = pP.tile([128, 128], F32)
            make_identity(nc, ident[:])

            dg_raw = pP.tile([128, G], F32)
            nc.sync.dma_start(out=dg_raw[:], in_=degp_g[:, :])
            dis_g = pP.tile([128, G], F32)
            nc.vector.reciprocal(dis_g[:], dg_raw[:])
            nc.scalar.activation(dis_g[:], dis_g[:], AF.Sqrt)

            dl_raw = pP.tile([128, GL], F32)
            nc.sync.dma_start(out=dl_raw[:], in_=degp_l[:, :])
            dis_l = pP.tile([128, GL], F32)
            nc.vector.reciprocal(dis_l[:], dl_raw[:])
            nc.scalar.activation(dis_l[:], dis_l[:], AF.Sqrt)

            # ---- zero table tails (gather views may touch them)
            with tc.tile_pool(name="zero", bufs=1) as pZ:
                ZCOLS = 4096
                zsb = pZ.tile([128, ZCOLS], F32)
                nc.vector.memset(zsb[:], 0.0)

                def zero_flat(flat_ap, n_elems, dt):
                    off = 0
                    zv = zsb[:].bitcast(dt) if dt != F32 else zsb[:]
                    percall = 128 * (ZCOLS * 2 if dt == BF16 else ZCOLS)
                    while off < n_elems:
                        f = min(percall, n_elems - off) // 128
                        nc.sync.dma_start(
                            out=flat_ap[off:off + 128 * f].rearrange(
                                "(p f) -> p f", f=f),
                            in_=zv[:, 0:f],
                        )
                        off += 128 * f

                zero_flat(hn1f.ap()[NP * HID:], 256, BF16)
                zero_flat(hn2f.ap()[NP * OUT:], 256, BF16)
                if d["NTAB2F"] > NP * OUT:
                    zero_flat(z2_t.ap()[NP * OUT:], d["NTAB2F"] - NP * OUT, F32)

            # ---- phase A: full hn1 table (bf16, redundant on every core)
            hn1_r = hn1f.ap()[0:NP * HID].rearrange(
                "(g t p e) -> g p t e", t=XB, p=128, e=HID)
            with tc.tile_pool(name="stream", bufs=3) as pS, tc.tile_pool(
                name="psumA", bufs=4, space="PSUM"
            ) as psA:
                for blk in range(G // XB):
                    xt = pS.tile([128, XB * FEAT], BF16, tag="xt")
                    nc.sync.dma_start(
                        out=xt[:], in_=xT[:, blk * XB * FEAT:(blk + 1) * XB * FEAT]
                    )
                    hn_sb = pS.tile([128, XB * HID], BF16, tag="hn")
                    for t in range(XB):
                        ps = psA.tile([128, HID], F32)
                        nc.tensor.matmul(
                            ps[:],
                            lhsT=xt[:, t * 128:(t + 1) * 128],
                            rhs=w1_sb[:],
                            start=True,
                            stop=True,
                        )
                        g = blk * XB + t
                        nc.vector.tensor_scalar_mul(
                            hn_sb[:, t * HID:(t + 1) * HID], ps[:],
                            dis_g[:, g:g + 1]
                        )
                    nc.sync.dma_start(
                        out=hn1_r[blk],
                        in_=hn_sb[:].rearrange("p (t e) -> p t e", e=HID))

            # ---- layer aggregation loops
            def layer_phase(plan, gidx_in, dstc_in, nch, view_of_run, width,
                            epilogue, pMsg, pOh, psAgg):
                dcol = pP.tile([128, nch], F32, tag=f"dcol{width}")
                nc.sync.dma_start(out=dcol[:], in_=dstc_in[:, :])
                ci_base = 0
                for pgrp in plan:
                    goff = pgrp["goff"]
                    mtiles = {}
                    for r, off, n in pgrp["pieces"]:
                        gi = pIdx.tile([128, n // 16], I16, tag="gi")
                        nc.sync.dma_start(
                            out=gi[:],
                            in_=gidx_in[:, (goff + off) // 16:
                                        (goff + off + n) // 16],
                        )
                        msg = pMsg.tile([128, n // 128, 128], BF16, tag="msg")
                        nc.gpsimd.dma_gather(
                            msg[:], view_of_run(r), gi[:], n, n, 128,
                            single_packet=False,
                        )
                        base = off // 128
                        for t in range(n // 128):
                            mtiles[(r, base + t)] = (msg, t)
                    # consumption: g-major chunk list
                    chunks = pgrp["chunks"]
                    # build one-hots in batches of 8
                    ohs = {}
                    for bstart in range(0, len(chunks), 8):
                        bn = min(8, len(chunks) - bstart)
                        oh8 = pOh.tile([128, 8, 128], BF16, tag="oh8")
                        nc.vector.tensor_tensor(
                            out=oh8[:, 0:bn, :],
                            in0=dcol[:, ci_base + bstart:ci_base + bstart + bn,
                                     None].to_broadcast([128, bn, 128]),
                            in1=iota[:, None, :].to_broadcast([128, bn, 128]),
                            op=ALU.is_equal,
                        )
                        for k in range(bn):
                            ohs[bstart + k] = (oh8, k)
                    cur_g = None
                    agg = None
                    for ci, (g, r, tcol) in enumerate(chunks):
                        if g != cur_g:
                            if cur_g is not None:
                                epilogue(cur_g, agg)
                            cur_g = g
                            agg = psAgg.tile([128, width], F32, tag="agg")
                            first = True
                        msg, t = mtiles[(r, tcol)]
                        oh8, k = ohs[ci]
                        last = (ci + 1 == len(chunks)) or chunks[ci + 1][0] != g
                        nc.tensor.matmul(
                            agg[:],
                            lhsT=oh8[:, k, :],
                            rhs=msg[:, t, 0:width],
                            start=first,
                            stop=last,
                        )
                        first = False
                    epilogue(cur_g, agg)
                    ci_base += len(chunks)

            # ---- layer 1 epilogue: z=relu(agg*dis+b1); hn2=(zT@W2)*dis
            hn2_r = hn2_sh.ap().rearrange("(g p e) -> g p e", p=128, e=OUT)

            with (
                tc.tile_pool(name="msg1", bufs=7) as pMsg,
                tc.tile_pool(name="oh1", bufs=4) as pOh,
                tc.tile_pool(name="agg1", bufs=3, space="PSUM") as psAgg,
                tc.tile_pool(name="epi1", bufs=3) as pEpi,
                tc.tile_pool(name="psepi1", bufs=2, space="PSUM") as psEpi,
            ):
                def epi1(g, agg):
                    zt = pEpi.tile([128, HID], F32, tag="zt")
                    nc.vector.tensor_scalar_mul(zt[:], agg[:], dis_l[:, g:g + 1])
                    nc.vector.tensor_tensor(out=zt[:], in0=zt[:], in1=b1_sb[:],
                                            op=ALU.add)
                    nc.scalar.activation(zt[:], zt[:], AF.Relu)
                    ps_zT = psEpi.tile([64, 128], F32, tag="pszt")
                    nc.tensor.transpose(ps_zT[:], zt[:], ident[:])
                    zT_sb = pEpi.tile([64, 128], BF16, tag="ztT")
                    nc.vector.tensor_copy(zT_sb[:], ps_zT[:])
                    ps_h2 = psEpi.tile([128, OUT], F32, tag="psh2")
                    nc.tensor.matmul(ps_h2[:], lhsT=zT_sb[:], rhs=w2_sb[:],
                                     start=True, stop=True)
                    hn2 = pEpi.tile([128, OUT], BF16, tag="hn2")
                    nc.vector.tensor_scalar_mul(hn2[:], ps_h2[:],
                                                dis_l[:, g:g + 1])
                    nc.sync.dma_start(out=hn2_r[g], in_=hn2[:])

                layer_phase(plan1, gidx1, dstc1, nch1, l1_view, HID, epi1,
                            pMsg, pOh, psAgg)

            nc.gpsimd.collective_compute(
                "AllGather",
                ALU.bypass,
                replica_groups=groups,
                ins=[hn2_sh.ap()],
                outs=[hn2f.ap()[0:NP * OUT]],
            )

            # ---- layer 2 epilogue: z2 = agg2*dis + b2
            z2_r = z2_sh.ap().rearrange("(g p e) -> g p e", p=128, e=OUT)

            with (
                tc.tile_pool(name="msg2", bufs=11) as pMsg2,
                tc.tile_pool(name="oh2", bufs=4) as pOh2,
                tc.tile_pool(name="agg2", bufs=3, space="PSUM") as psAgg2,
                tc.tile_pool(name="epi2", bufs=3) as pEpi2,
            ):
                def epi2(g, agg):
                    z2 = pEpi2.tile([128, OUT], F32, tag="z2")
                    nc.vector.tensor_scalar_mul(z2[:], agg[:], dis_l[:, g:g + 1])
                    nc.vector.tensor_tensor(out=z2[:], in0=z2[:], in1=b2_sb[:],
                                            op=ALU.add)
                    nc.sync.dma_start(out=z2_r[g], in_=z2[:])

                layer_phase(plan2, gidx2, dstc2, nch2, l2_view, OUT, epi2,
                            pMsg2, pOh2, psAgg2)

            nc.gpsimd.collective_compute(
                "AllGather",
                ALU.bypass,
                replica_groups=groups,
                ins=[z2_sh.ap()],
                outs=[z2_t.ap()[0:NP * OUT]],
            )

            # ---- final: edge logits (v1 machinery)
            with tc.tile_pool(name="fin", bufs=3) as pFin:
                colsF = TILE_F // 16
                for s in range(16):
                    for t in range(n_ft):
                        off16 = (s * n_ft + t) * colsF
                        fa = pIdx.tile([128, colsF], I16, tag="fa")
                        nc.sync.dma_start(
                            out=fa[:], in_=fidxA[:, off16:off16 + colsF])
                        fb = pIdx.tile([128, colsF], I16, tag="fb")
                        nc.sync.dma_start(
                            out=fb[:], in_=fidxB[:, off16:off16 + colsF])
                        ma = pFin.tile([128, TJ_F, HID], F32, tag="ma")
                        nc.gpsimd.dma_gather(
                            ma[:], ftab_view(s >> 2), fa[:], TILE_F, TILE_F,
                            HID, single_packet=False,
                        )
                        mb = pFin.tile([128, TJ_F, HID], F32, tag="mb")
                        nc.gpsimd.dma_gather(
                            mb[:], ftab_view(s & 3), fb[:], TILE_F, TILE_F,
                            HID, single_packet=False,
                        )
                        prod = pFin.tile([128, TJ_F, OUT], F32, tag="prod")
                        nc.vector.tensor_tensor(
                            out=prod[:],
                            in0=ma[:, :, 0:OUT],
                            in1=mb[:, :, 0:OUT],
                            op=ALU.mult,
                        )
                        red = pFin.tile([128, TJ_F], F32, tag="red")
                        nc.vector.reduce_sum(
                            out=red[:, :, None],
                            in_=prod[:],
                            axis=mybir.AxisListType.X,
                        )
                        blk = s * n_ft + t
                        nc.sync.dma_start(
                            out=lraw.ap()[blk * TILE_F:(blk + 1) * TILE_F]
                            .rearrange("(p j) -> p j", j=TJ_F),
                            in_=red[:],
                        )

    nc.compile()
    return nc


# ---------------------------------------------------------------- entry point

_CACHE = {}
TRACE = False
LAST = {}


def kernel(**inputs):
    cfg = derive(default_cfg())
    in_maps, meta = prep_host(inputs, cfg)
    key = (meta["plan1"], meta["plan2"], meta["n_ft"])
    if key not in _CACHE:
        _CACHE[key] = build(cfg, meta)
    nc = _CACHE[key]
    res = bass_utils.run_bass_kernel_spmd(
        nc, in_maps, core_ids=list(range(cfg["C"])), trace=TRACE
    )
    LAST["res"] = res
    return assemble(res.results, meta, cfg)
